# revision 23
# baseline (speedup 1.0000x reference)
"""Trainium2 Bass kernel for CrossModalRefinementCell (cell_id != 0,3 branch).

Computation (D=1024, BS=256):
    h        = relu(text @ aw1 + ab1)                  [BS, D]
    attn     = softmax(h @ aw2 + ab2, axis=1)          [BS, D]
    t        = text * attn                             [BS, D]
    pre_txt  = t @ rw1[D:]                             [BS, D]
    pre_img  = image @ rw1[:D]                         [BS, D]
    hid[i,j] = relu(pre_txt[i] + pre_img[j] + rb1)     [BS, BS, D]
    res[i,j] = image[j] + hid[i,j] @ rw2 + rb2         [BS, BS, D]

Sharding: data-parallel over the outer text index i -- each of the 8 cores
gets 32 text rows (sliced on host), all weights + image replicated. Each
core emits out[32, 256, 1024]; host concatenates along axis 0.

On-device layout: everything i-indexed is computed in "transposed space"
(d on partitions, i on the free dim) so that per-i values become
per-partition bias columns, and hid^T tiles [d_blk(128), j(256)] feed the
main matmul as the stationary operand: out[j,dout] = hidT.T @ rw2.
"""

import os
import sys

sys.path.insert(0, "/opt/trn_rl_repo")
os.environ.setdefault("MYCRO_LOCAL_CACHE", "1")

import numpy as np

import concourse.bacc as bacc
import concourse.bass as bass
import concourse.mybir as mybir
import concourse.tile as tile
from concourse.bass_utils import run_bass_kernel_spmd

D = 1024
BS = 256
NCORES = 8
IPC = BS // NCORES  # 32 text rows per core
KB = D // 128  # 8 k-blocks of 128

F32 = mybir.dt.float32
AF = mybir.ActivationFunctionType
ALU = mybir.AluOpType
AX = mybir.AxisListType

# dtype of the main-loop (pairwise) matmul: "float32" or "bfloat16".
# bf16 runs the PE at 1 cyc/row (4x fp32) with norm-rel error ~6e-4.
MM_DTYPE = getattr(mybir.dt, os.environ.get("MM_DTYPE", "bfloat16"))


def _mm_ap(ap):
    return ap


def build():
    nc = bacc.Bacc(
        "TRN2",
        target_bir_lowering=False,
        debug=False,
        enable_asserts=False,
        num_devices=NCORES,
    )

    BF = MM_DTYPE  # bf16 for all weight matrices (halves DMA, 1 cyc/row PE)
    text_sl = nc.dram_tensor("text_sl", [IPC, D], F32, kind="ExternalInput")
    image = nc.dram_tensor("image", [BS, D], F32, kind="ExternalInput")
    aw1 = nc.dram_tensor("aw1", [D, D], BF, kind="ExternalInput")
    aw2 = nc.dram_tensor("aw2", [D, D], BF, kind="ExternalInput")
    rw1i = nc.dram_tensor("rw1i", [D, D], BF, kind="ExternalInput")
    rw1t = nc.dram_tensor("rw1t", [D, D], BF, kind="ExternalInput")
    rw2 = nc.dram_tensor("rw2", [D, D], BF, kind="ExternalInput")
    # biases: column layouts [128, KB] (col k = k-th 128-block); broadcast rows
    ab1c = nc.dram_tensor("ab1c", [128, KB], F32, kind="ExternalInput")
    ab2b = nc.dram_tensor("ab2b", [IPC, D], F32, kind="ExternalInput")
    rb1c = nc.dram_tensor("rb1c", [128, KB], F32, kind="ExternalInput")
    rb2b = nc.dram_tensor("rb2b", [128, D], F32, kind="ExternalInput")
    out = nc.dram_tensor("out", [IPC, BS, D], F32, kind="ExternalOutput")

    ident_d = nc.inline_tensor(np.eye(128, dtype=np.float32), "ident_d")

    with tile.TileContext(nc) as tc:
        with tc.tile_pool(name="persist", bufs=1) as pp:
            # ---- persistent tiles (live through the main loop) ----
            ident = pp.tile([128, 128], F32)
            rw2_sb = [pp.tile([128, D], BF, name=f"rw2_{k}") for k in range(KB)]
            B_sb = pp.tile([128, KB * BS], F32)  # pre_imgT + rb1, blk k at k*256
            ptxT_sb = pp.tile([128, KB * IPC], F32)  # pre_txtT, blk k at k*32
            ir_sb = [pp.tile([128, D], F32, name=f"ir_{j}") for j in range(2)]
            rb1c_sb = pp.tile([128, KB], F32)
            ab1c_sb = pp.tile([128, KB], F32)

            nc.sync.dma_start(ident[:], ident_d[:])
            nc.sync.dma_start(rb1c_sb[:], rb1c[:])
            nc.sync.dma_start(ab1c_sb[:], ab1c[:])

            # ---- setup-scoped tiles ----
            from contextlib import ExitStack
            from itertools import cycle

            setup_ctx = ExitStack()
            wp = setup_ctx.enter_context(tc.tile_pool(name="wpool", bufs=32))
            sp = setup_ctx.enter_context(tc.tile_pool(name="setup", bufs=1))

            # small critical tensors first so they land ahead of the weights
            text_sb = sp.tile([IPC, D], F32)
            nc.sync.dma_start(text_sb[:], text_sl[:])
            ab2b_sb = sp.tile([IPC, D], F32)
            rb2b_sb = sp.tile([128, D], F32)
            nc.gpsimd.dma_start(ab2b_sb[:], ab2b[:])
            nc.gpsimd.dma_start(rb2b_sb[:], rb2b[:])

            # spread weight loads across engine DMA queues for parallelism
            dma_engines = cycle([nc.sync, nc.gpsimd, nc.scalar])

            def load_mat(dram, tag):
                tiles = []
                for k in range(KB):
                    t = wp.tile([128, D], BF, name=f"{tag}{k}", tag="w")
                    next(dma_engines).dma_start(t[:], dram[k * 128 : (k + 1) * 128, :])
                    tiles.append(t)
                return tiles

            aw1_sb = load_mat(aw1, "aw1_")
            aw2_sb = load_mat(aw2, "aw2_")
            rw1i_sb = load_mat(rw1i, "rw1i_")
            rw1t_sb = load_mat(rw1t, "rw1t_")

            image_sb = []
            for j in range(2):
                t = sp.tile([128, D], F32, name=f"image_{j}")
                next(dma_engines).dma_start(t[:], image[j * 128 : (j + 1) * 128, :])
                image_sb.append(t)

            for k in range(KB):
                next(dma_engines).dma_start(rw2_sb[k][:], rw2[k * 128 : (k + 1) * 128, :])

            textT_sb = sp.tile([128, KB * IPC], BF)
            hT_sb = sp.tile([128, KB * IPC], BF)
            logits_sb = sp.tile([IPC, D], F32)
            e_sb = sp.tile([IPC, D], F32)
            ta_sb = sp.tile([IPC, D], F32)
            taT_sb = sp.tile([128, KB * IPC], BF)
            imgT_sb = sp.tile([128, KB * BS], BF)
            negmax = sp.tile([IPC, 1], F32)
            ssum = sp.tile([IPC, 1], F32)
            rsum = sp.tile([IPC, 1], F32)

            with tc.tile_pool(name="psetup", bufs=4, space="PSUM") as pps:
                # textT: transpose text_sl [32, 1024] -> [128, 32] x KB
                for k in range(KB):
                    ps = pps.tile([128, IPC], F32, tag="ps", name=f"psT{k}")
                    nc.tensor.transpose(
                        ps[:], text_sb[:, k * 128 : (k + 1) * 128], ident[0:IPC, 0:IPC]
                    )
                    nc.vector.tensor_copy(
                        textT_sb[:, k * IPC : (k + 1) * IPC], ps[:]
                    )

                # hT[dh, i] = relu(aw1.T @ textT + ab1)
                for dh in range(KB):
                    ps = pps.tile([128, IPC], F32, tag="ps", name=f"psh{dh}")
                    for k in range(KB):
                        nc.tensor.matmul(
                            ps[:],
                            _mm_ap(aw1_sb[k][:, dh * 128 : (dh + 1) * 128]),
                            _mm_ap(textT_sb[:, k * IPC : (k + 1) * IPC]),
                            start=(k == 0),
                            stop=(k == KB - 1),
                        )
                    nc.scalar.activation(
                        hT_sb[:, dh * IPC : (dh + 1) * IPC],
                        ps[:],
                        AF.Relu,
                        bias=ab1c_sb[:, dh : dh + 1],
                    )

                # logits[i, dl] = hT.T @ aw2 + ab2 (row space for softmax)
                for dlb in range(2):
                    ps = pps.tile([IPC, 512], F32, tag="ps", name=f"psl{dlb}")
                    for dh in range(KB):
                        nc.tensor.matmul(
                            ps[:],
                            hT_sb[:, dh * IPC : (dh + 1) * IPC],
                            aw2_sb[dh][:, dlb * 512 : (dlb + 1) * 512],
                            start=(dh == 0),
                            stop=(dh == KB - 1),
                        )
                    nc.vector.tensor_add(
                        logits_sb[:, dlb * 512 : (dlb + 1) * 512],
                        ps[:],
                        ab2b_sb[:, dlb * 512 : (dlb + 1) * 512],
                    )

                # softmax over the feature (free) dim
                nc.vector.tensor_reduce(
                    negmax[:], logits_sb[:], axis=AX.X, op=ALU.max, negate=True
                )
                nc.scalar.activation(
                    e_sb[:], logits_sb[:], AF.Exp,
                    bias=negmax[:, 0:1], accum_out=ssum[:],
                )
                nc.vector.reciprocal(rsum[:], ssum[:])
                # t = text * attn = text * e * (1/sum)
                nc.vector.tensor_mul(ta_sb[:], e_sb[:], text_sb[:])
                nc.vector.tensor_scalar(
                    ta_sb[:], ta_sb[:], rsum[:, 0:1], None, op0=ALU.mult
                )

                # taT: transpose t
                for k in range(KB):
                    ps = pps.tile([128, IPC], F32, tag="ps", name=f"psta{k}")
                    nc.tensor.transpose(
                        ps[:], ta_sb[:, k * 128 : (k + 1) * 128], ident[0:IPC, 0:IPC]
                    )
                    nc.vector.tensor_copy(taT_sb[:, k * IPC : (k + 1) * IPC], ps[:])

                # pre_txtT[d, i] = rw1t.T @ taT
                for db in range(KB):
                    ps = pps.tile([128, IPC], F32, tag="ps", name=f"pspt{db}")
                    for k in range(KB):
                        nc.tensor.matmul(
                            ps[:],
                            _mm_ap(rw1t_sb[k][:, db * 128 : (db + 1) * 128]),
                            _mm_ap(taT_sb[:, k * IPC : (k + 1) * IPC]),
                            start=(k == 0),
                            stop=(k == KB - 1),
                        )
                    nc.vector.tensor_copy(
                        ptxT_sb[:, db * IPC : (db + 1) * IPC], ps[:]
                    )

                # imgT: transpose image [256, 1024] -> blocks [128, 256]
                for k in range(KB):
                    for j in range(2):
                        ps = pps.tile([128, 128], F32, tag="ps", name=f"psi{k}_{j}")
                        nc.tensor.transpose(
                            ps[:], image_sb[j][:, k * 128 : (k + 1) * 128], ident[:]
                        )
                        nc.vector.tensor_copy(
                            imgT_sb[:, k * BS + j * 128 : k * BS + (j + 1) * 128],
                            ps[:],
                        )

                # B[d, j] = rw1i.T @ imgT + rb1
                for db in range(KB):
                    ps = pps.tile([128, BS], F32, tag="ps", name=f"psB{db}")
                    for k in range(KB):
                        nc.tensor.matmul(
                            ps[:],
                            _mm_ap(rw1i_sb[k][:, db * 128 : (db + 1) * 128]),
                            _mm_ap(imgT_sb[:, k * BS : (k + 1) * BS]),
                            start=(k == 0),
                            stop=(k == KB - 1),
                        )
                    nc.vector.tensor_scalar(
                        B_sb[:, db * BS : (db + 1) * BS],
                        ps[:],
                        rb1c_sb[:, db : db + 1],
                        None,
                        op0=ALU.add,
                    )

                # ir[j, dout] = image + rb2 (exact fp32 adds, no PE)
                for j in range(2):
                    nc.vector.tensor_add(ir_sb[j][:], image_sb[j][:], rb2b_sb[:])

            setup_ctx.close()  # release wpool/setup SBUF before the main loop

            # ---- main loop over this core's 32 text rows ----
            with (
                tc.tile_pool(name="hid", bufs=2) as hp,
                tc.tile_pool(name="outp", bufs=6) as op_,
                tc.tile_pool(name="pmain", bufs=8, space="PSUM") as pm,
            ):
                for i in range(IPC):
                    hidT = hp.tile([128, KB * BS], MM_DTYPE, name="hidT", tag="hidT")
                    for db in range(KB):
                        nc.scalar.activation(
                            hidT[:, db * BS : (db + 1) * BS],
                            B_sb[:, db * BS : (db + 1) * BS],
                            AF.Relu,
                            bias=ptxT_sb[:, db * IPC + i : db * IPC + i + 1],
                        )
                    for jb in range(2):
                        for db2 in range(2):
                            ps = pm.tile([128, 512], F32, tag="pmm", name="pmm")
                            for db in range(KB):
                                nc.tensor.matmul(
                                    ps[:],
                                    _mm_ap(
                                        hidT[
                                            :,
                                            db * BS + jb * 128 : db * BS + (jb + 1) * 128,
                                        ]
                                    ),
                                    _mm_ap(rw2_sb[db][:, db2 * 512 : (db2 + 1) * 512]),
                                    start=(db == 0),
                                    stop=(db == KB - 1),
                                )
                            o = op_.tile([128, 512], F32, name="o", tag="o")
                            nc.vector.tensor_add(
                                o[:], ps[:], ir_sb[jb][:, db2 * 512 : (db2 + 1) * 512]
                            )
                            nc.sync.dma_start(
                                out[
                                    i,
                                    jb * 128 : (jb + 1) * 128,
                                    db2 * 512 : (db2 + 1) * 512,
                                ],
                                o[:],
                            )
    nc.compile()
    return nc


_NC_CACHE = None


def _get_nc():
    global _NC_CACHE
    if _NC_CACHE is None:
        _NC_CACHE = build()
    return _NC_CACHE


def _make_in_maps(inputs):
    import ml_dtypes

    f32 = np.float32
    bf = ml_dtypes.bfloat16
    text = np.ascontiguousarray(np.asarray(inputs["text_features"], f32))
    image = np.ascontiguousarray(np.asarray(inputs["image_features"], f32))
    aw1 = np.ascontiguousarray(np.asarray(inputs["aw1"], f32).astype(bf))
    aw2 = np.ascontiguousarray(np.asarray(inputs["aw2"], f32).astype(bf))
    rw1 = np.asarray(inputs["rw1"], f32)
    rw1i = np.ascontiguousarray(rw1[:D].astype(bf))
    rw1t = np.ascontiguousarray(rw1[D:].astype(bf))
    rw2 = np.ascontiguousarray(np.asarray(inputs["rw2"], f32).astype(bf))

    def col(b):  # [D] -> [128, KB]
        return np.ascontiguousarray(np.asarray(b, f32).reshape(KB, 128).T)

    ab2 = np.asarray(inputs["ab2"], f32).reshape(1, D)
    rb2 = np.asarray(inputs["rb2"], f32).reshape(1, D)
    shared = {
        "image": image, "aw1": aw1, "aw2": aw2,
        "rw1i": rw1i, "rw1t": rw1t, "rw2": rw2,
        "ab1c": col(inputs["ab1"]), "rb1c": col(inputs["rb1"]),
        "ab2b": np.ascontiguousarray(np.broadcast_to(ab2, (IPC, D))),
        "rb2b": np.ascontiguousarray(np.broadcast_to(rb2, (128, D))),
    }
    return [
        {**shared, "text_sl": np.ascontiguousarray(text[c * IPC : (c + 1) * IPC])}
        for c in range(NCORES)
    ]


def _run(inputs, **kwargs):
    cell_id = int(np.asarray(inputs["cell_id"]))
    assert cell_id not in (0, 3), f"cell_id={cell_id} branch not implemented"
    nc = _get_nc()
    res = run_bass_kernel_spmd(nc, _make_in_maps(inputs), list(range(NCORES)), **kwargs)
    full = np.concatenate([res.results[c]["out"] for c in range(NCORES)], axis=0)
    return full, res


def kernel(**inputs) -> np.ndarray:
    full, _ = _run(inputs)
    return full
